# revision 4
# baseline (speedup 1.0000x reference)
"""Distributed Bass kernel for nn_Attention_6287832122083 on 8 TRN2 NeuronCores.

Strategy (v2): tensor-parallel over heads (2 heads per core) with
token-sharded output projection.
 - All matmul operands are bf16 (FWL fast weight loads; fp32 PSUM accum).
 - Each core computes q,k,v for its 2 heads; RoPE is fused into the
   PSUM->SBUF drain (cos/sin multiply against PSUM, partition-half swap
   via gpsimd copies).
 - Attention per (batch, head, query-chunk) with causal tile skipping;
   the causal diagonal is handled by multiplying the post-exp tile by a
   [128,128] upper-triangular 0/1 matrix on the vector engine (no mask
   preload matmuls).  Row sums accumulate on DVE in fp32; normalization
   via ones-matmul broadcast + fast reciprocal.
 - Attention outputs are redistributed with 4 small AllToAlls (one per
   (batch, half-sequence)), giving each core all heads for a 128-token
   slice per (batch,half).  Stage 2 computes outT = gathered.T @ woT
   with the gathered tile stationary (few LDWEIGHTS, N=512 moving).
 - Emission interleaves qkv chunks, attention units and stage-2 pieces
   so PE fills every exp/DVE dependency gap.
"""

import math
from contextlib import ExitStack

import numpy as np

import concourse.bass as bass
import concourse.bacc as bacc
import concourse.mybir as mybir
import concourse.tile as tile
from concourse import bass_utils

F32 = mybir.dt.float32
BF16 = mybir.dt.bfloat16
NP_BF16 = mybir.dt.np(BF16)
EXP = mybir.ActivationFunctionType.Exp

B, S, D, H = 2, 2048, 2048, 16
HD = D // H              # 128
T = B * S                # 4096 tokens
NCORES = 8
HLOC = H // NCORES       # 2 heads per core
CHK = 512                # qkv token chunk == attention query chunk
NCHK_B = S // CHK        # 4 chunks per batch
KT = 128                 # key tile
NDT = D // 128           # 16 contraction tiles
SCALE = 1.0 / math.sqrt(HD)


def build_kernel():
    nc = bacc.Bacc(
        "TRN2",
        target_bir_lowering=False,
        debug=False,
        enable_asserts=False,
        num_devices=NCORES,
    )

    # ---- per-core DRAM parameters (host pre-tiled, bf16) ----
    xt = nc.dram_tensor("xt", [128, B * NCHK_B, NDT, CHK], BF16, kind="ExternalInput")
    wqp = nc.dram_tensor("wqp", [128, NDT, 256], BF16, kind="ExternalInput")
    wkp = nc.dram_tensor("wkp", [128, NDT, 256], BF16, kind="ExternalInput")
    wvp = nc.dram_tensor("wvp", [128, NDT, 256], BF16, kind="ExternalInput")
    wop = nc.dram_tensor("wop", [128, NDT, D], BF16, kind="ExternalInput")
    cs1 = nc.dram_tensor("cs1", [128, S], BF16, kind="ExternalInput")
    cs2 = nc.dram_tensor("cs2", [128, S], BF16, kind="ExternalInput")
    trid = nc.dram_tensor("trid", [128, 128], BF16, kind="ExternalInput")
    onesd = nc.dram_tensor("onesd", [128, 128], BF16, kind="ExternalInput")
    # out rows: b*256 + half*128 + t  <->  full[b, half*1024 + c*128 + t, :]
    out = nc.dram_tensor("out", [2 * B * 128, D], F32, kind="ExternalOutput")

    with tile.TileContext(nc) as tc:
        with ExitStack() as stack:
            # ---------------- pools ----------------
            const_pool = stack.enter_context(tc.tile_pool(name="const", bufs=1))
            wo_pool = stack.enter_context(tc.tile_pool(name="wop", bufs=1))
            w_pool = stack.enter_context(tc.tile_pool(name="wpool", bufs=1))
            x_pool = stack.enter_context(tc.tile_pool(name="xc", bufs=2))
            qkv_pool = stack.enter_context(tc.tile_pool(name="qkv", bufs=2))
            rope_pool = stack.enter_context(tc.tile_pool(name="rope", bufs=1))
            pt_pool = stack.enter_context(tc.tile_pool(name="ptp", bufs=3))
            acc_pool = stack.enter_context(tc.tile_pool(name="accp", bufs=2))
            small_pool = stack.enter_context(tc.tile_pool(name="smallp", bufs=2))
            s2g_pool = stack.enter_context(tc.tile_pool(name="s2g", bufs=1))
            ost_pool = stack.enter_context(tc.tile_pool(name="ostp", bufs=1))

            ps_qk = stack.enter_context(tc.tile_pool(name="psqk", bufs=2, space="PSUM"))
            ps_v = stack.enter_context(tc.tile_pool(name="psv", bufs=1, space="PSUM"))
            ps_sc = stack.enter_context(tc.tile_pool(name="pssc", bufs=2, space="PSUM"))
            ps_pv = stack.enter_context(tc.tile_pool(name="pspv", bufs=2, space="PSUM"))
            ps_s2 = stack.enter_context(tc.tile_pool(name="pss2", bufs=1, space="PSUM"))

            dram_pool = stack.enter_context(
                tc.tile_pool(name="dram", bufs=1, space="DRAM")
            )

            # ---------------- persistent tiles ----------------
            cs1_sb = const_pool.tile([128, S], BF16, name="cs1_sb")
            cs2_sb = const_pool.tile([128, S], BF16, name="cs2_sb")
            tri_sb = const_pool.tile([128, 128], BF16, name="tri_sb")
            ones_sb = const_pool.tile([128, 128], BF16, name="ones_sb")
            wo_sb = wo_pool.tile([128, NDT, D], BF16, name="wo_sb")
            wq_sb = w_pool.tile([128, NDT, 256], BF16, name="wq_sb")
            wk_sb = w_pool.tile([128, NDT, 256], BF16, name="wk_sb")
            wv_sb = w_pool.tile([128, NDT, 256], BF16, name="wv_sb")

            a2a_in = [
                [
                    dram_pool.tile([NCORES, HLOC, 128, 128], BF16, name=f"ain{b}_{hf}")
                    for hf in range(2)
                ]
                for b in range(B)
            ]
            a2a_out = [
                [
                    dram_pool.tile([NCORES, HLOC, 128, 128], BF16, name=f"aout{b}_{hf}")
                    for hf in range(2)
                ]
                for b in range(B)
            ]

            # ---------------- qkv tiles (rotate per batch) ----------------
            cur = {}

            def open_qkv(b):
                cur[b] = (
                    qkv_pool.tile([128, HLOC, S], BF16, tag="q", name=f"q{b}"),
                    qkv_pool.tile([128, HLOC, S], BF16, tag="k", name=f"k{b}"),
                    qkv_pool.tile([128, S // KT, 256], BF16, tag="v", name=f"v{b}"),
                )

            # ---------------- x chunk DMAs ----------------
            chunk_list = [(b, ch) for b in range(B) for ch in range(NCHK_B)]
            x_tiles = {}

            def issue_x_dma(idx, split=2):
                b, ch = chunk_list[idx]
                t = x_pool.tile([128, NDT, CHK], BF16, tag="xc", name=f"x{b}_{ch}")
                src = xt.ap()[:, b * NCHK_B + ch]
                step = NDT // split
                for s in range(split):
                    sl = slice(s * step, (s + 1) * step)
                    nc.sync.dma_start(t[:, sl, :], src[:, sl, :])
                x_tiles[idx] = t

            def emit_initial_dmas():
                # critical path first: wq/x0 in fine pieces, then wk, wv
                issue_x_dma(0, split=8)
                q4 = NDT // 4
                for s in range(4):
                    sl = slice(s * q4, (s + 1) * q4)
                    nc.sync.dma_start(wq_sb[:, sl, :], wqp.ap()[:, sl, :])
                nc.sync.dma_start(wk_sb[:], wkp.ap())
                nc.sync.dma_start(wv_sb[:], wvp.ap())
                nc.sync.dma_start(cs1_sb[:], cs1.ap())
                nc.sync.dma_start(cs2_sb[:], cs2.ap())
                nc.sync.dma_start(tri_sb[:], trid.ap())
                nc.sync.dma_start(ones_sb[:], onesd.ap())
                for s in range(4):
                    sl = slice(s * q4, (s + 1) * q4)
                    nc.sync.dma_start(wo_sb[:, sl, :], wop.ap()[:, sl, :])

            # ---------------- qkv pieces ----------------
            def rope_drain(ps, dst, h, sl):
                # dst[:, h, sl] = rope(ps) with cs1=[cos;-sin], cs2=[sin;cos]
                t1 = rope_pool.tile([128, CHK], F32, tag="t1")
                t2 = rope_pool.tile([128, CHK], F32, tag="t2")
                t1s = rope_pool.tile([64, CHK], F32, tag="t1s")
                t2s = rope_pool.tile([64, CHK], F32, tag="t2s")
                nc.vector.tensor_mul(t1[:], ps[:], cs1_sb[:, sl])
                nc.vector.tensor_mul(t2[:], ps[:], cs2_sb[:, sl])
                nc.gpsimd.tensor_copy(t1s[:], t1[64:128, :])
                nc.gpsimd.tensor_copy(t2s[:], t2[64:128, :])
                nc.vector.tensor_add(dst[0:64, h, sl], t1[0:64, :], t1s[:])
                nc.vector.tensor_add(dst[64:128, h, sl], t2[0:64, :], t2s[:])

            def emit_qkv_qk(b, ch, h):
                idx = b * NCHK_B + ch
                xc = x_tiles[idx]
                q_sb, k_sb, _ = cur[b]
                sl = slice(ch * CHK, (ch + 1) * CHK)
                psq = ps_qk.tile([128, CHK], F32, tag="psqk")
                for dt in range(NDT):
                    nc.tensor.matmul(
                        psq[:],
                        lhsT=wq_sb[:, dt, h * HD : (h + 1) * HD],
                        rhs=xc[:, dt, :],
                        start=(dt == 0),
                        stop=(dt == NDT - 1),
                    )
                psk = ps_qk.tile([128, CHK], F32, tag="psqk")
                for dt in range(NDT):
                    nc.tensor.matmul(
                        psk[:],
                        lhsT=wk_sb[:, dt, h * HD : (h + 1) * HD],
                        rhs=xc[:, dt, :],
                        start=(dt == 0),
                        stop=(dt == NDT - 1),
                    )
                rope_drain(psq, q_sb, h, sl)
                rope_drain(psk, k_sb, h, sl)

            def emit_qkv_v(b, ch):
                idx = b * NCHK_B + ch
                xc = x_tiles[idx]
                _, _, v_sb = cur[b]
                for st in range(CHK // KT):
                    psv = ps_v.tile([128, 256], F32, tag="psv")
                    for dt in range(NDT):
                        nc.tensor.matmul(
                            psv[:],
                            lhsT=xc[:, dt, st * KT : (st + 1) * KT],
                            rhs=wv_sb[:, dt, :],
                            start=(dt == 0),
                            stop=(dt == NDT - 1),
                        )
                    nc.vector.tensor_copy(
                        v_sb[:, ch * (CHK // KT) + st, :], psv[:]
                    )

            # ---------------- attention ----------------
            pending = []

            def flush_tail():
                while pending:
                    pending.pop(0)()

            def emit_attn(b, h, tcq):
                q_sb, k_sb, v_sb = cur[b]
                q0 = tcq * CHK
                nkt = (tcq + 1) * (CHK // KT)
                pv = ps_pv.tile([128, CHK], F32, tag="pv")
                acc = acc_pool.tile([128, CHK], F32, tag="acc")
                pt0 = None
                for kt in range(nkt):
                    k0 = kt * KT
                    j = kt - (CHK // KT) * tcq
                    off = KT * j if j >= 0 else 0
                    ps = ps_sc.tile([128, CHK], F32, tag="sc")
                    nc.tensor.matmul(
                        ps[:, off:],
                        lhsT=k_sb[:, h, k0 : k0 + KT],
                        rhs=q_sb[:, h, q0 + off : q0 + CHK],
                        start=True,
                        stop=True,
                    )
                    pt = pt_pool.tile([128, CHK], BF16, tag="pt")
                    nc.scalar.activation(pt[:, off:], ps[:, off:], EXP, scale=SCALE)
                    if j >= 0:
                        nc.vector.tensor_mul(
                            pt[:, off : off + KT], pt[:, off : off + KT], tri_sb[:]
                        )
                    if kt == 0:
                        pt0 = pt
                    elif kt == 1:
                        if tcq == 0:
                            nc.vector.tensor_add(
                                acc[:, KT:], pt0[:, KT:], pt[:, KT:]
                            )
                            nc.vector.tensor_copy(acc[:, 0:KT], pt0[:, 0:KT])
                        else:
                            nc.vector.tensor_add(acc[:], pt0[:], pt[:])
                    else:
                        nc.vector.tensor_add(
                            acc[:, off:], acc[:, off:], pt[:, off:]
                        )
                    nc.tensor.matmul(
                        pv[:, off:],
                        lhsT=v_sb[:, kt, h * HD : (h + 1) * HD],
                        rhs=pt[:, off:],
                        start=(kt == 0),
                        stop=(kt == nkt - 1),
                    )
                    if kt == 0:
                        flush_tail()

                def tail():
                    accb = small_pool.tile([128, CHK], BF16, tag="accb")
                    nc.vector.tensor_copy(accb[:], acc[:])
                    lb = ps_sc.tile([128, CHK], F32, tag="sc")
                    nc.tensor.matmul(
                        lb[:], lhsT=ones_sb[:], rhs=accb[:], start=True, stop=True
                    )
                    rbs = small_pool.tile([128, CHK], F32, tag="rbs")
                    nc.vector.reciprocal_approx_fast(rbs[:], lb[:])
                    aon = small_pool.tile([128, CHK], BF16, tag="aon")
                    nc.vector.tensor_mul(aon[:], pv[:], rbs[:])
                    dstt = a2a_in[b][tcq // 2]
                    for i in range(4):
                        d = 4 * (tcq % 2) + i
                        nc.sync.dma_start(
                            dstt[d, h], aon[:, i * KT : (i + 1) * KT]
                        )

                pending.append(tail)

            def emit_a2a(b, hf):
                flush_tail()
                nc.gpsimd.collective_compute(
                    "AllToAll",
                    mybir.AluOpType.bypass,
                    replica_groups=[list(range(NCORES))],
                    ins=[a2a_in[b][hf].opt()],
                    outs=[a2a_out[b][hf].opt()],
                )

            # ---------------- stage 2 ----------------
            gt_tiles = {}

            def load_gt(b, hf):
                l = []
                for ad in range(NDT):
                    g = s2g_pool.tile(
                        [128, 128], BF16, tag="gt", bufs=20, name=f"g{b}{hf}{ad}"
                    )
                    nc.sync.dma_start(g[:], a2a_out[b][hf][ad // 2, ad % 2])
                    l.append(g)
                gt_tiles[(b, hf)] = l

            def emit_s2_piece(b, hf, oc):
                gts = gt_tiles[(b, hf)]
                ps2 = ps_s2.tile([128, 512], F32, tag="s2")
                for ad in range(NDT):
                    nc.tensor.matmul(
                        ps2[:],
                        lhsT=gts[ad][:],
                        rhs=wo_sb[:, ad, oc * 512 : (oc + 1) * 512],
                        start=(ad == 0),
                        stop=(ad == NDT - 1),
                    )
                ost = ost_pool.tile([128, 512], F32, tag="ost")
                nc.vector.tensor_copy(ost[:], ps2[:])
                r0 = b * 256 + hf * 128
                nc.sync.dma_start(
                    out.ap()[r0 : r0 + 128, oc * 512 : (oc + 1) * 512], ost[:]
                )

            # ---------------- schedule ----------------
            emit_initial_dmas()
            open_qkv(0)
            s2_sched = [(0, 0, oc) for oc in range(4)] + [(0, 1, oc) for oc in range(4)]
            for i, (b, ch) in enumerate(chunk_list):
                if ch == 0:
                    if b == 1:
                        open_qkv(1)
                        load_gt(0, 0)
                emit_qkv_qk(b, ch, 0)
                if i + 1 < len(chunk_list):
                    issue_x_dma(i + 1)
                emit_qkv_qk(b, ch, 1)
                emit_qkv_v(b, ch)
                if b == 1 and ch == 2:
                    load_gt(0, 1)
                emit_attn(b, 0, ch)
                if b == 1:
                    emit_s2_piece(*s2_sched[2 * ch])
                emit_attn(b, 1, ch)
                if b == 1:
                    emit_s2_piece(*s2_sched[2 * ch + 1])
                if ch == 1:
                    emit_a2a(b, 0)
                elif ch == 3:
                    emit_a2a(b, 1)
            flush_tail()
            load_gt(1, 0)
            for oc in range(4):
                emit_s2_piece(1, 0, oc)
            load_gt(1, 1)
            for oc in range(4):
                emit_s2_piece(1, 1, oc)

    nc.compile()
    return nc


_CACHED = {}


def _get_compiled():
    if "nc" not in _CACHED:
        _CACHED["nc"] = build_kernel()
    return _CACHED["nc"]


def shard_inputs(x, wq, wk, wv, wo, freqs_cos, freqs_sin, mask):
    x = np.asarray(x, np.float32)
    wq = np.asarray(wq, np.float32)
    wk = np.asarray(wk, np.float32)
    wv = np.asarray(wv, np.float32)
    wo = np.asarray(wo, np.float32)
    fc = np.asarray(freqs_cos, np.float32)
    fs = np.asarray(freqs_sin, np.float32)

    X = x.reshape(T, D).T  # [D, T]
    xt = (
        np.ascontiguousarray(X.reshape(NDT, 128, B * NCHK_B, CHK).transpose(1, 2, 0, 3))
        .astype(NP_BF16)
    )  # [128, 8, 16, 512]

    # de-interleave within each head: [0,2,...,126, 1,3,...,127]
    perm = np.concatenate([np.arange(0, HD, 2), np.arange(1, HD, 2)])

    ct = fc.T  # [64, S]
    st = fs.T
    cs1 = np.ascontiguousarray(np.concatenate([ct, -st], axis=0)).astype(NP_BF16)
    cs2 = np.ascontiguousarray(np.concatenate([st, ct], axis=0)).astype(NP_BF16)
    tri = np.triu(np.ones((128, 128), np.float32)).astype(NP_BF16)
    ones = np.ones((128, 128), np.float32).astype(NP_BF16)

    def tile_w(w_c):  # [256 out, D in] -> [128, NDT, 256]
        return np.ascontiguousarray(
            w_c.T.reshape(NDT, 128, 256).transpose(1, 0, 2)
        ).astype(NP_BF16)

    woT = wo.T  # [d_in, d_out]
    wot = np.ascontiguousarray(woT.reshape(NDT, 128, D).transpose(1, 0, 2)).astype(
        NP_BF16
    )

    in_maps = []
    for c in range(NCORES):
        rows = slice(c * HLOC * HD, (c + 1) * HLOC * HD)
        wq_c = wq[rows].reshape(HLOC, HD, D)[:, perm, :].reshape(HLOC * HD, D)
        wk_c = wk[rows].reshape(HLOC, HD, D)[:, perm, :].reshape(HLOC * HD, D)
        wv_c = wv[rows]
        in_maps.append(
            {
                "xt": xt,
                "wqp": tile_w(wq_c),
                "wkp": tile_w(wk_c),
                "wvp": tile_w(wv_c),
                "wop": wot,
                "cs1": cs1,
                "cs2": cs2,
                "trid": tri,
                "onesd": ones,
            }
        )
    return in_maps


def run_sharded(in_maps, trace=False):
    nc = _get_compiled()
    res = bass_utils.run_bass_kernel_spmd(
        nc, in_maps, core_ids=list(range(NCORES)), trace=trace
    )
    return res


def unshard(results):
    # results: list of dicts with "out": [512, D]; row = b*256 + half*128 + t
    A = np.stack([r["out"] for r in results])  # [8, 512, 2048]
    full = A.reshape(NCORES, B, 2, 128, D).transpose(1, 2, 0, 3, 4)
    return np.ascontiguousarray(full.reshape(B, S, D))


def kernel(**inputs):
    in_maps = shard_inputs(**inputs)
    res = run_sharded(in_maps, trace=False)
    return unshard(res.results)


# revision 15
# speedup vs baseline: 1.1079x; 1.1079x over previous
"""Distributed Bass kernel for nn_Attention_6287832122083 on 8 TRN2 NeuronCores.

Strategy (v2): tensor-parallel over heads (2 heads per core) with
token-sharded output projection.
 - All matmul operands are bf16 (FWL fast weight loads; fp32 PSUM accum).
 - Each core computes q,k,v for its 2 heads; RoPE is fused into the
   PSUM->SBUF drain (cos/sin multiply against PSUM, partition-half swap
   via gpsimd copies).
 - Attention per (batch, head, query-chunk) with causal tile skipping;
   the causal diagonal is handled by multiplying the post-exp tile by a
   [128,128] upper-triangular 0/1 matrix on the vector engine (no mask
   preload matmuls).  Row sums accumulate on DVE in fp32; normalization
   via ones-matmul broadcast + fast reciprocal.
 - Attention outputs are redistributed with 4 small AllToAlls (one per
   (batch, half-sequence)), giving each core all heads for a 128-token
   slice per (batch,half).  Stage 2 computes outT = gathered.T @ woT
   with the gathered tile stationary (few LDWEIGHTS, N=512 moving).
 - Emission interleaves qkv chunks, attention units and stage-2 pieces
   so PE fills every exp/DVE dependency gap.
"""

import math
from contextlib import ExitStack

import numpy as np

import concourse.bass as bass
import concourse.bacc as bacc
import concourse.mybir as mybir
import concourse.tile as tile
from concourse import bass_utils

F32 = mybir.dt.float32
BF16 = mybir.dt.bfloat16
NP_BF16 = mybir.dt.np(BF16)
EXP = mybir.ActivationFunctionType.Exp

B, S, D, H = 2, 2048, 2048, 16
HD = D // H              # 128
T = B * S                # 4096 tokens
NCORES = 8
HLOC = H // NCORES       # 2 heads per core
CHK = 512                # qkv token chunk == attention query chunk
NCHK_B = S // CHK        # 4 chunks per batch
KT = 128                 # key tile
NDT = D // 128           # 16 contraction tiles
SCALE = 1.0 / math.sqrt(HD)


def build_kernel():
    nc = bacc.Bacc(
        "TRN2",
        target_bir_lowering=False,
        debug=False,
        enable_asserts=False,
        num_devices=NCORES,
    )

    # ---- per-core DRAM parameters (host pre-tiled, bf16) ----
    xt = nc.dram_tensor("xt", [128, B * NCHK_B, NDT, CHK], BF16, kind="ExternalInput")
    wqp = nc.dram_tensor("wqp", [128, NDT, 256], BF16, kind="ExternalInput")
    wkp = nc.dram_tensor("wkp", [128, NDT, 256], BF16, kind="ExternalInput")
    wvp = nc.dram_tensor("wvp", [128, NDT, 256], BF16, kind="ExternalInput")
    wop = nc.dram_tensor("wop", [128, NDT, D], BF16, kind="ExternalInput")
    cs1 = nc.dram_tensor("cs1", [128, S], BF16, kind="ExternalInput")
    cs2 = nc.dram_tensor("cs2", [128, S], BF16, kind="ExternalInput")
    trid = nc.dram_tensor("trid", [128, 128], BF16, kind="ExternalInput")
    onesd = nc.dram_tensor("onesd", [128, 128], BF16, kind="ExternalInput")
    # out rows: b*256 + half*128 + t  <->  full[b, half*1024 + c*128 + t, :]
    out = nc.dram_tensor("out", [2 * B * 128, D], F32, kind="ExternalOutput")

    with tile.TileContext(nc) as tc:
        with ExitStack() as stack:
            # ---------------- pools ----------------
            const_pool = stack.enter_context(tc.tile_pool(name="const", bufs=1))
            wo_pool = stack.enter_context(tc.tile_pool(name="wop", bufs=1))
            w_pool = stack.enter_context(tc.tile_pool(name="wpool", bufs=1))
            x_pool = stack.enter_context(tc.tile_pool(name="xc", bufs=2))
            qkv_pool = stack.enter_context(tc.tile_pool(name="qkv", bufs=2))
            rope_pool = stack.enter_context(tc.tile_pool(name="rope", bufs=1))
            pt_pool = stack.enter_context(tc.tile_pool(name="ptp", bufs=3))
            acc_pool = stack.enter_context(tc.tile_pool(name="accp", bufs=2))
            small_pool = stack.enter_context(tc.tile_pool(name="smallp", bufs=2))
            s2g_pool = stack.enter_context(tc.tile_pool(name="s2g", bufs=1))
            ost_pool = stack.enter_context(tc.tile_pool(name="ostp", bufs=1))

            ps_qk = stack.enter_context(tc.tile_pool(name="psqk", bufs=2, space="PSUM"))
            ps_v = stack.enter_context(tc.tile_pool(name="psv", bufs=1, space="PSUM"))
            ps_sc = stack.enter_context(tc.tile_pool(name="pssc", bufs=2, space="PSUM"))
            ps_pv = stack.enter_context(tc.tile_pool(name="pspv", bufs=2, space="PSUM"))
            ps_s2 = stack.enter_context(tc.tile_pool(name="pss2", bufs=1, space="PSUM"))

            dram_pool = stack.enter_context(
                tc.tile_pool(name="dram", bufs=1, space="DRAM")
            )

            # ---------------- persistent tiles ----------------
            cs1_sb = const_pool.tile([128, S], BF16, name="cs1_sb")
            cs2_sb = const_pool.tile([128, S], BF16, name="cs2_sb")
            tri_sb = const_pool.tile([128, 128], BF16, name="tri_sb")
            ones_sb = const_pool.tile([128, 128], BF16, name="ones_sb")
            wo_sb = wo_pool.tile([128, NDT, D], BF16, name="wo_sb")
            wq_sb = w_pool.tile([128, NDT, 256], BF16, name="wq_sb")
            wk_sb = w_pool.tile([128, NDT, 256], BF16, name="wk_sb")
            wv_sb = w_pool.tile([128, NDT, 256], BF16, name="wv_sb")

            a2a_in = [
                [
                    dram_pool.tile([NCORES, HLOC, 128, 128], BF16, name=f"ain{b}_{hf}")
                    for hf in range(2)
                ]
                for b in range(B)
            ]
            a2a_out = [
                [
                    dram_pool.tile([NCORES, HLOC, 128, 128], BF16, name=f"aout{b}_{hf}")
                    for hf in range(2)
                ]
                for b in range(B)
            ]

            # ---------------- qkv tiles (rotate per batch) ----------------
            cur = {}

            def open_qkv(b):
                cur[b] = (
                    qkv_pool.tile([128, HLOC, S], BF16, tag="q", name=f"q{b}"),
                    qkv_pool.tile([128, HLOC, S], BF16, tag="k", name=f"k{b}"),
                    qkv_pool.tile([128, S // KT, 256], BF16, tag="v", name=f"v{b}"),
                )

            # ---------------- x chunk DMAs ----------------
            chunk_list = [(b, ch) for b in range(B) for ch in range(NCHK_B)]
            x_tiles = {}

            def issue_x_dma(idx, split=2):
                b, ch = chunk_list[idx]
                t = x_pool.tile([128, NDT, CHK], BF16, tag="xc", name=f"x{b}_{ch}")
                src = xt.ap()[:, b * NCHK_B + ch]
                step = NDT // split
                for s in range(split):
                    sl = slice(s * step, (s + 1) * step)
                    nc.sync.dma_start(t[:, sl, :], src[:, sl, :])
                x_tiles[idx] = t

            def emit_initial_dmas():
                # critical-path-interleaved: first q-matmul needs wq[0:4]+x0[0:4];
                # k chain follows ~3.4us later, rope drains need cs1/cs2, v needs wv.
                b0 = xt.ap()[:, 0]
                xc0 = x_pool.tile([128, NDT, CHK], BF16, tag="xc", name="x0_0")
                x_tiles[0] = xc0
                nc.sync.dma_start(wq_sb[:, 0:4, :], wqp.ap()[:, 0:4, :])
                nc.sync.dma_start(xc0[:, 0:4, :], b0[:, 0:4, :])
                nc.sync.dma_start(wk_sb[:, 0:8, :], wkp.ap()[:, 0:8, :])
                nc.sync.dma_start(xc0[:, 4:8, :], b0[:, 4:8, :])
                nc.sync.dma_start(wq_sb[:, 4:16, :], wqp.ap()[:, 4:16, :])
                nc.sync.dma_start(cs1_sb[:], cs1.ap())
                nc.sync.dma_start(xc0[:, 8:12, :], b0[:, 8:12, :])
                nc.sync.dma_start(cs2_sb[:], cs2.ap())
                nc.sync.dma_start(wk_sb[:, 8:16, :], wkp.ap()[:, 8:16, :])
                nc.sync.dma_start(xc0[:, 12:16, :], b0[:, 12:16, :])
                nc.sync.dma_start(wv_sb[:], wvp.ap())
                nc.sync.dma_start(tri_sb[:], trid.ap())
                nc.sync.dma_start(ones_sb[:], onesd.ap())

            # wo loads in 2-tile pieces spread across the chunk loop so the
            # 8.4MB never head-of-line-blocks an x-chunk prefetch
            wo_state = {"next": 0}

            def emit_wo_piece(n=2):
                s = wo_state["next"]
                if s >= NDT:
                    return
                nc.sync.dma_start(
                    wo_sb[:, s : s + n, :], wop.ap()[:, s : s + n, :]
                )
                wo_state["next"] = s + n

            # ---------------- qkv pieces ----------------
            def rope_drain(ps, dst, h, sl):
                # dst[:, h, sl] = rope(ps) with cs1=[cos;-sin], cs2=[sin;cos]
                t1 = rope_pool.tile([128, CHK], BF16, tag="t1")
                t2 = rope_pool.tile([128, CHK], BF16, tag="t2")
                t1s = rope_pool.tile([64, CHK], BF16, tag="t1s")
                t2s = rope_pool.tile([64, CHK], BF16, tag="t2s")
                nc.vector.tensor_mul(t1[:], ps[:], cs1_sb[:, sl])
                nc.vector.tensor_mul(t2[:], ps[:], cs2_sb[:, sl])
                nc.scalar.copy(t1s[:], t1[64:128, :])
                nc.scalar.copy(t2s[:], t2[64:128, :])
                nc.vector.tensor_add(dst[0:64, h, sl], t1[0:64, :], t1s[:])
                nc.vector.tensor_add(dst[64:128, h, sl], t2[0:64, :], t2s[:])

            def emit_qkv_qk(b, ch, h):
                idx = b * NCHK_B + ch
                xc = x_tiles[idx]
                q_sb, k_sb, _ = cur[b]
                sl = slice(ch * CHK, (ch + 1) * CHK)
                psq = ps_qk.tile([128, CHK], F32, tag="psqk")
                for dt in range(NDT):
                    nc.tensor.matmul(
                        psq[:],
                        lhsT=wq_sb[:, dt, h * HD : (h + 1) * HD],
                        rhs=xc[:, dt, :],
                        start=(dt == 0),
                        stop=(dt == NDT - 1),
                    )
                psk = ps_qk.tile([128, CHK], F32, tag="psqk")
                for dt in range(NDT):
                    nc.tensor.matmul(
                        psk[:],
                        lhsT=wk_sb[:, dt, h * HD : (h + 1) * HD],
                        rhs=xc[:, dt, :],
                        start=(dt == 0),
                        stop=(dt == NDT - 1),
                    )
                rope_drain(psq, q_sb, h, sl)
                rope_drain(psk, k_sb, h, sl)

            def emit_qkv_v(b, ch):
                idx = b * NCHK_B + ch
                xc = x_tiles[idx]
                _, _, v_sb = cur[b]
                for st in range(CHK // KT):
                    psv = ps_v.tile([128, 256], F32, tag="psv")
                    for dt in range(NDT):
                        nc.tensor.matmul(
                            psv[:],
                            lhsT=xc[:, dt, st * KT : (st + 1) * KT],
                            rhs=wv_sb[:, dt, :],
                            start=(dt == 0),
                            stop=(dt == NDT - 1),
                        )
                    nc.vector.tensor_copy(
                        v_sb[:, ch * (CHK // KT) + st, :], psv[:]
                    )

            # ---------------- attention ----------------
            pending = []

            def flush_tail():
                while pending:
                    pending.pop(0)()

            def emit_attn(b, h, tcq):
                q_sb, k_sb, v_sb = cur[b]
                q0 = tcq * CHK
                nkt = (tcq + 1) * (CHK // KT)
                pv = ps_pv.tile([128, CHK], F32, tag="pv")
                acc = acc_pool.tile([128, CHK], F32, tag="acc")
                pt0 = None
                for kt in range(nkt):
                    k0 = kt * KT
                    j = kt - (CHK // KT) * tcq
                    off = KT * j if j >= 0 else 0
                    ps = ps_sc.tile([128, CHK], F32, tag="sc")
                    nc.tensor.matmul(
                        ps[:, off:],
                        lhsT=k_sb[:, h, k0 : k0 + KT],
                        rhs=q_sb[:, h, q0 + off : q0 + CHK],
                        start=True,
                        stop=True,
                    )
                    pt = pt_pool.tile([128, CHK], BF16, tag="pt")
                    nc.scalar.activation(pt[:, off:], ps[:, off:], EXP, scale=SCALE)
                    if j >= 0:
                        nc.vector.tensor_mul(
                            pt[:, off : off + KT], pt[:, off : off + KT], tri_sb[:]
                        )
                    if kt == 0:
                        pt0 = pt
                    elif kt == 1:
                        if tcq == 0:
                            nc.vector.tensor_add(
                                acc[:, KT:], pt0[:, KT:], pt[:, KT:]
                            )
                            nc.vector.tensor_copy(acc[:, 0:KT], pt0[:, 0:KT])
                        else:
                            nc.vector.tensor_add(acc[:], pt0[:], pt[:])
                    else:
                        nc.vector.tensor_add(
                            acc[:, off:], acc[:, off:], pt[:, off:]
                        )
                    nc.tensor.matmul(
                        pv[:, off:],
                        lhsT=v_sb[:, kt, h * HD : (h + 1) * HD],
                        rhs=pt[:, off:],
                        start=(kt == 0),
                        stop=(kt == nkt - 1),
                    )
                    if kt == 0:
                        flush_tail()

                def tail():
                    accb = small_pool.tile([128, CHK], BF16, tag="accb")
                    nc.vector.tensor_copy(accb[:], acc[:])
                    lb = ps_sc.tile([128, CHK], F32, tag="sc")
                    nc.tensor.matmul(
                        lb[:], lhsT=ones_sb[:], rhs=accb[:], start=True, stop=True
                    )
                    rbs = small_pool.tile([128, CHK], F32, tag="rbs")
                    nc.vector.reciprocal_approx_fast(rbs[:], lb[:])
                    aon = small_pool.tile([128, CHK], BF16, tag="aon")
                    nc.vector.tensor_mul(aon[:], pv[:], rbs[:])
                    dstt = a2a_in[b][tcq // 2]
                    for i in range(4):
                        d = 4 * (tcq % 2) + i
                        nc.sync.dma_start(
                            dstt[d, h], aon[:, i * KT : (i + 1) * KT]
                        )

                pending.append(tail)

            def emit_a2a(b, hf):
                flush_tail()
                nc.gpsimd.collective_compute(
                    "AllToAll",
                    mybir.AluOpType.bypass,
                    replica_groups=[list(range(NCORES))],
                    ins=[a2a_in[b][hf].opt()],
                    outs=[a2a_out[b][hf].opt()],
                )

            # ---------------- stage 2 ----------------
            gt_tiles = {}

            def load_gt(b, hf):
                # one bulk DMA on gpsimd, right after the producing collective:
                # ordering on the gpsimd stream guarantees it waits exactly for
                # its own A2A, and PE only waits on this DMA's semaphore.
                g = s2g_pool.tile(
                    [128, NDT, 128], BF16, tag="gt", bufs=2, name=f"g{b}{hf}"
                )
                src = a2a_out[b][hf][:].rearrange("s l p t -> p (s l) t")
                nc.gpsimd.dma_start(g[:], src)
                gt_tiles[(b, hf)] = g

            def emit_s2_piece(b, hf, oc):
                g = gt_tiles[(b, hf)]
                ps2 = ps_s2.tile([128, 512], F32, tag="s2")
                for ad in range(NDT):
                    nc.tensor.matmul(
                        ps2[:],
                        lhsT=g[:, ad, :],
                        rhs=wo_sb[:, ad, oc * 512 : (oc + 1) * 512],
                        start=(ad == 0),
                        stop=(ad == NDT - 1),
                    )
                ost = ost_pool.tile([128, 512], F32, tag="ost")
                nc.vector.tensor_copy(ost[:], ps2[:])
                r0 = b * 256 + hf * 128
                nc.sync.dma_start(
                    out.ap()[r0 : r0 + 128, oc * 512 : (oc + 1) * 512], ost[:]
                )

            # ---------------- schedule ----------------
            emit_initial_dmas()
            open_qkv(0)
            s2_sched = [(0, 0, oc) for oc in range(4)] + [(0, 1, oc) for oc in range(4)]
            for i, (b, ch) in enumerate(chunk_list):
                if b == 1 and ch == 0:
                    open_qkv(1)
                if i + 1 < len(chunk_list):
                    issue_x_dma(i + 1)
                emit_wo_piece()
                emit_qkv_qk(b, ch, 0)
                emit_wo_piece()
                emit_qkv_qk(b, ch, 1)
                emit_qkv_v(b, ch)
                emit_attn(b, 0, ch)
                if b == 1:
                    emit_s2_piece(*s2_sched[2 * ch])
                emit_attn(b, 1, ch)
                if b == 1:
                    emit_s2_piece(*s2_sched[2 * ch + 1])
                if ch == 1:
                    emit_a2a(b, 0)
                    load_gt(b, 0)
                elif ch == 3:
                    emit_a2a(b, 1)
                    load_gt(b, 1)
            # tail: s2(1,0) pieces cover the last A2A's flight
            for oc in range(4):
                emit_s2_piece(1, 0, oc)
            for oc in range(4):
                emit_s2_piece(1, 1, oc)

    nc.compile()
    return nc


_CACHED = {}


def _get_compiled():
    if "nc" not in _CACHED:
        _CACHED["nc"] = build_kernel()
    return _CACHED["nc"]


def shard_inputs(x, wq, wk, wv, wo, freqs_cos, freqs_sin, mask):
    x = np.asarray(x, np.float32)
    wq = np.asarray(wq, np.float32)
    wk = np.asarray(wk, np.float32)
    wv = np.asarray(wv, np.float32)
    wo = np.asarray(wo, np.float32)
    fc = np.asarray(freqs_cos, np.float32)
    fs = np.asarray(freqs_sin, np.float32)

    X = x.reshape(T, D).T  # [D, T]
    xt = (
        np.ascontiguousarray(X.reshape(NDT, 128, B * NCHK_B, CHK).transpose(1, 2, 0, 3))
        .astype(NP_BF16)
    )  # [128, 8, 16, 512]

    # de-interleave within each head: [0,2,...,126, 1,3,...,127]
    perm = np.concatenate([np.arange(0, HD, 2), np.arange(1, HD, 2)])

    ct = fc.T  # [64, S]
    st = fs.T
    cs1 = np.ascontiguousarray(np.concatenate([ct, -st], axis=0)).astype(NP_BF16)
    cs2 = np.ascontiguousarray(np.concatenate([st, ct], axis=0)).astype(NP_BF16)
    tri = np.triu(np.ones((128, 128), np.float32)).astype(NP_BF16)
    ones = np.ones((128, 128), np.float32).astype(NP_BF16)

    def tile_w(w_c):  # [256 out, D in] -> [128, NDT, 256]
        return np.ascontiguousarray(
            w_c.T.reshape(NDT, 128, 256).transpose(1, 0, 2)
        ).astype(NP_BF16)

    woT = wo.T  # [d_in, d_out]
    wot = np.ascontiguousarray(woT.reshape(NDT, 128, D).transpose(1, 0, 2)).astype(
        NP_BF16
    )

    in_maps = []
    for c in range(NCORES):
        rows = slice(c * HLOC * HD, (c + 1) * HLOC * HD)
        wq_c = wq[rows].reshape(HLOC, HD, D)[:, perm, :].reshape(HLOC * HD, D)
        wk_c = wk[rows].reshape(HLOC, HD, D)[:, perm, :].reshape(HLOC * HD, D)
        wv_c = wv[rows]
        in_maps.append(
            {
                "xt": xt,
                "wqp": tile_w(wq_c),
                "wkp": tile_w(wk_c),
                "wvp": tile_w(wv_c),
                "wop": wot,
                "cs1": cs1,
                "cs2": cs2,
                "trid": tri,
                "onesd": ones,
            }
        )
    return in_maps


def run_sharded(in_maps, trace=False):
    nc = _get_compiled()
    res = bass_utils.run_bass_kernel_spmd(
        nc, in_maps, core_ids=list(range(NCORES)), trace=trace
    )
    return res


def unshard(results):
    # results: list of dicts with "out": [512, D]; row = b*256 + half*128 + t
    A = np.stack([r["out"] for r in results])  # [8, 512, 2048]
    full = A.reshape(NCORES, B, 2, 128, D).transpose(1, 2, 0, 3, 4)
    return np.ascontiguousarray(full.reshape(B, S, D))


def kernel(**inputs):
    in_maps = shard_inputs(**inputs)
    res = run_sharded(in_maps, trace=False)
    return unshard(res.results)


# revision 19
# speedup vs baseline: 1.1996x; 1.0828x over previous
"""Distributed Bass kernel for nn_Attention_6287832122083 on 8 TRN2 NeuronCores.

Strategy (v2): tensor-parallel over heads (2 heads per core) with
token-sharded output projection.
 - All matmul operands are bf16 (FWL fast weight loads; fp32 PSUM accum).
 - Each core computes q,k,v for its 2 heads; RoPE is fused into the
   PSUM->SBUF drain (cos/sin multiply against PSUM, partition-half swap
   via gpsimd copies).
 - Attention per (batch, head, query-chunk) with causal tile skipping;
   the causal diagonal is handled by multiplying the post-exp tile by a
   [128,128] upper-triangular 0/1 matrix on the vector engine (no mask
   preload matmuls).  Row sums accumulate on DVE in fp32; normalization
   via ones-matmul broadcast + fast reciprocal.
 - Attention outputs are redistributed with 4 small AllToAlls (one per
   (batch, half-sequence)), giving each core all heads for a 128-token
   slice per (batch,half).  Stage 2 computes outT = gathered.T @ woT
   with the gathered tile stationary (few LDWEIGHTS, N=512 moving).
 - Emission interleaves qkv chunks, attention units and stage-2 pieces
   so PE fills every exp/DVE dependency gap.
"""

import math
from contextlib import ExitStack

import numpy as np

import concourse.bass as bass
import concourse.bacc as bacc
import concourse.mybir as mybir
import concourse.tile as tile
from concourse import bass_utils

F32 = mybir.dt.float32
BF16 = mybir.dt.bfloat16
NP_BF16 = mybir.dt.np(BF16)
EXP = mybir.ActivationFunctionType.Exp

B, S, D, H = 2, 2048, 2048, 16
HD = D // H              # 128
T = B * S                # 4096 tokens
NCORES = 8
HLOC = H // NCORES       # 2 heads per core
CHK = 512                # qkv token chunk == attention query chunk
NCHK_B = S // CHK        # 4 chunks per batch
KT = 128                 # key tile
NDT = D // 128           # 16 contraction tiles
SCALE = 1.0 / math.sqrt(HD)


def build_kernel():
    nc = bacc.Bacc(
        "TRN2",
        target_bir_lowering=False,
        debug=False,
        enable_asserts=False,
        num_devices=NCORES,
    )

    # ---- per-core DRAM parameters (host pre-tiled, bf16) ----
    xt = nc.dram_tensor("xt", [128, B * NCHK_B, NDT, CHK], BF16, kind="ExternalInput")
    wqp = nc.dram_tensor("wqp", [128, NDT, 256], BF16, kind="ExternalInput")
    wkp = nc.dram_tensor("wkp", [128, NDT, 256], BF16, kind="ExternalInput")
    wvp = nc.dram_tensor("wvp", [128, NDT, 256], BF16, kind="ExternalInput")
    wop = nc.dram_tensor("wop", [128, NDT, D], BF16, kind="ExternalInput")
    cs1 = nc.dram_tensor("cs1", [128, S], BF16, kind="ExternalInput")
    cs2 = nc.dram_tensor("cs2", [128, S], BF16, kind="ExternalInput")
    trid = nc.dram_tensor("trid", [128, 128], BF16, kind="ExternalInput")
    onesd = nc.dram_tensor("onesd", [128, 128], BF16, kind="ExternalInput")
    # out rows: b*256 + half*128 + t  <->  full[b, half*1024 + c*128 + t, :]
    out = nc.dram_tensor("out", [2 * B * 128, D], F32, kind="ExternalOutput")

    with tile.TileContext(nc) as tc:
        with ExitStack() as stack:
            # ---------------- pools ----------------
            const_pool = stack.enter_context(tc.tile_pool(name="const", bufs=1))
            wo_pool = stack.enter_context(tc.tile_pool(name="wop", bufs=1))
            w_pool = stack.enter_context(tc.tile_pool(name="wpool", bufs=1))
            x_pool = stack.enter_context(tc.tile_pool(name="xc", bufs=2))
            qkv_pool = stack.enter_context(tc.tile_pool(name="qkv", bufs=2))
            rope_pool = stack.enter_context(tc.tile_pool(name="rope", bufs=1))
            pt_pool = stack.enter_context(tc.tile_pool(name="ptp", bufs=3))
            acc_pool = stack.enter_context(tc.tile_pool(name="accp", bufs=2))
            small_pool = stack.enter_context(tc.tile_pool(name="smallp", bufs=2))
            s2g_pool = stack.enter_context(tc.tile_pool(name="s2g", bufs=1))
            ost_pool = stack.enter_context(tc.tile_pool(name="ostp", bufs=2))

            ps_qk = stack.enter_context(tc.tile_pool(name="psqk", bufs=2, space="PSUM"))
            ps_v = stack.enter_context(tc.tile_pool(name="psv", bufs=1, space="PSUM"))
            ps_sc = stack.enter_context(tc.tile_pool(name="pssc", bufs=2, space="PSUM"))
            ps_pv = stack.enter_context(tc.tile_pool(name="pspv", bufs=2, space="PSUM"))
            ps_s2 = stack.enter_context(tc.tile_pool(name="pss2", bufs=1, space="PSUM"))

            dram_pool = stack.enter_context(
                tc.tile_pool(name="dram", bufs=1, space="DRAM")
            )

            # ---------------- persistent tiles ----------------
            cs1_sb = const_pool.tile([128, S], BF16, name="cs1_sb")
            cs2_sb = const_pool.tile([128, S], BF16, name="cs2_sb")
            tri_sb = const_pool.tile([128, 128], BF16, name="tri_sb")
            ones_sb = const_pool.tile([128, 128], BF16, name="ones_sb")
            wo_sb = wo_pool.tile([128, NDT, D], BF16, name="wo_sb")
            wq_sb = w_pool.tile([128, NDT, 256], BF16, name="wq_sb")
            wk_sb = w_pool.tile([128, NDT, 256], BF16, name="wk_sb")
            wv_sb = w_pool.tile([128, NDT, 256], BF16, name="wv_sb")

            a2a_in = [
                [
                    dram_pool.tile([NCORES, HLOC, 128, 128], BF16, name=f"ain{b}_{hf}")
                    for hf in range(2)
                ]
                for b in range(B)
            ]
            a2a_out = [
                [
                    dram_pool.tile([NCORES, HLOC, 128, 128], BF16, name=f"aout{b}_{hf}")
                    for hf in range(2)
                ]
                for b in range(B)
            ]

            # ---------------- qkv tiles (rotate per batch) ----------------
            cur = {}

            def open_qkv(b):
                cur[b] = (
                    qkv_pool.tile([128, HLOC, S], BF16, tag="q", name=f"q{b}"),
                    qkv_pool.tile([128, HLOC, S], BF16, tag="k", name=f"k{b}"),
                    qkv_pool.tile([128, S // KT, 256], BF16, tag="v", name=f"v{b}"),
                )

            # ---------------- x chunk DMAs ----------------
            chunk_list = [(b, ch) for b in range(B) for ch in range(NCHK_B)]
            x_tiles = {}

            def issue_x_dma(idx, split=2):
                b, ch = chunk_list[idx]
                t = x_pool.tile([128, NDT, CHK], BF16, tag="xc", name=f"x{b}_{ch}")
                src = xt.ap()[:, b * NCHK_B + ch]
                step = NDT // split
                for s in range(split):
                    sl = slice(s * step, (s + 1) * step)
                    nc.sync.dma_start(t[:, sl, :], src[:, sl, :])
                x_tiles[idx] = t

            def emit_initial_dmas():
                # critical-path-interleaved: first q-matmul needs wq[0:4]+x0[0:4];
                # k chain follows ~3.4us later, rope drains need cs1/cs2, v needs wv.
                b0 = xt.ap()[:, 0]
                xc0 = x_pool.tile([128, NDT, CHK], BF16, tag="xc", name="x0_0")
                x_tiles[0] = xc0
                nc.sync.dma_start(wq_sb[:, 0:4, :], wqp.ap()[:, 0:4, :])
                nc.sync.dma_start(xc0[:, 0:4, :], b0[:, 0:4, :])
                nc.sync.dma_start(wk_sb[:, 0:8, :], wkp.ap()[:, 0:8, :])
                nc.sync.dma_start(xc0[:, 4:8, :], b0[:, 4:8, :])
                nc.sync.dma_start(wq_sb[:, 4:16, :], wqp.ap()[:, 4:16, :])
                nc.sync.dma_start(wv_sb[:, 0:8, :], wvp.ap()[:, 0:8, :])
                nc.sync.dma_start(xc0[:, 8:12, :], b0[:, 8:12, :])
                nc.sync.dma_start(cs1_sb[:], cs1.ap())
                nc.sync.dma_start(wk_sb[:, 8:16, :], wkp.ap()[:, 8:16, :])
                nc.sync.dma_start(xc0[:, 12:16, :], b0[:, 12:16, :])
                nc.sync.dma_start(cs2_sb[:], cs2.ap())
                nc.sync.dma_start(wv_sb[:, 8:16, :], wvp.ap()[:, 8:16, :])
                nc.sync.dma_start(tri_sb[:], trid.ap())
                nc.sync.dma_start(ones_sb[:], onesd.ap())

            # wo loads in 2-tile pieces spread across the chunk loop so the
            # 8.4MB never head-of-line-blocks an x-chunk prefetch
            wo_state = {"next": 0}

            def emit_wo_piece(n=2):
                s = wo_state["next"]
                if s >= NDT:
                    return
                nc.sync.dma_start(
                    wo_sb[:, s : s + n, :], wop.ap()[:, s : s + n, :]
                )
                wo_state["next"] = s + n

            # ---------------- qkv pieces ----------------
            def rope_drain(ps, dst, h, sl):
                # dst[:, h, sl] = rope(ps) with cs1=[cos;-sin], cs2=[sin;cos]
                t1 = rope_pool.tile([128, CHK], BF16, tag="t1")
                t2 = rope_pool.tile([128, CHK], BF16, tag="t2")
                t1s = rope_pool.tile([64, CHK], BF16, tag="t1s")
                t2s = rope_pool.tile([64, CHK], BF16, tag="t2s")
                nc.vector.tensor_mul(t1[:], ps[:], cs1_sb[:, sl])
                nc.vector.tensor_mul(t2[:], ps[:], cs2_sb[:, sl])
                nc.scalar.copy(t1s[:], t1[64:128, :])
                nc.scalar.copy(t2s[:], t2[64:128, :])
                nc.vector.tensor_add(dst[0:64, h, sl], t1[0:64, :], t1s[:])
                nc.vector.tensor_add(dst[64:128, h, sl], t2[0:64, :], t2s[:])

            def emit_qkv_qk(b, ch, h):
                idx = b * NCHK_B + ch
                xc = x_tiles[idx]
                q_sb, k_sb, _ = cur[b]
                sl = slice(ch * CHK, (ch + 1) * CHK)
                psq = ps_qk.tile([128, CHK], F32, tag="psqk")
                for dt in range(NDT):
                    nc.tensor.matmul(
                        psq[:],
                        lhsT=wq_sb[:, dt, h * HD : (h + 1) * HD],
                        rhs=xc[:, dt, :],
                        start=(dt == 0),
                        stop=(dt == NDT - 1),
                    )
                psk = ps_qk.tile([128, CHK], F32, tag="psqk")
                for dt in range(NDT):
                    nc.tensor.matmul(
                        psk[:],
                        lhsT=wk_sb[:, dt, h * HD : (h + 1) * HD],
                        rhs=xc[:, dt, :],
                        start=(dt == 0),
                        stop=(dt == NDT - 1),
                    )
                rope_drain(psq, q_sb, h, sl)
                rope_drain(psk, k_sb, h, sl)

            def emit_qkv_v(b, ch):
                idx = b * NCHK_B + ch
                xc = x_tiles[idx]
                _, _, v_sb = cur[b]
                for st in range(CHK // KT):
                    psv = ps_v.tile([128, 256], F32, tag="psv")
                    for dt in range(NDT):
                        nc.tensor.matmul(
                            psv[:],
                            lhsT=xc[:, dt, st * KT : (st + 1) * KT],
                            rhs=wv_sb[:, dt, :],
                            start=(dt == 0),
                            stop=(dt == NDT - 1),
                        )
                    nc.vector.tensor_copy(
                        v_sb[:, ch * (CHK // KT) + st, :], psv[:]
                    )

            # ---------------- attention ----------------
            pending = []

            def flush_tail():
                while pending:
                    pending.pop(0)()

            def emit_attn(b, h, tcq):
                q_sb, k_sb, v_sb = cur[b]
                q0 = tcq * CHK
                nkt = (tcq + 1) * (CHK // KT)
                pv = ps_pv.tile([128, CHK], F32, tag="pv")
                acc = acc_pool.tile([128, CHK], F32, tag="acc")
                pt0 = None
                for kt in range(nkt):
                    k0 = kt * KT
                    j = kt - (CHK // KT) * tcq
                    off = KT * j if j >= 0 else 0
                    ps = ps_sc.tile([128, CHK], F32, tag="sc")
                    nc.tensor.matmul(
                        ps[:, off:],
                        lhsT=k_sb[:, h, k0 : k0 + KT],
                        rhs=q_sb[:, h, q0 + off : q0 + CHK],
                        start=True,
                        stop=True,
                    )
                    pt = pt_pool.tile([128, CHK], BF16, tag="pt")
                    nc.scalar.activation(pt[:, off:], ps[:, off:], EXP, scale=SCALE)
                    if j >= 0:
                        nc.vector.tensor_mul(
                            pt[:, off : off + KT], pt[:, off : off + KT], tri_sb[:]
                        )
                    if kt == 0:
                        pt0 = pt
                    elif kt == 1:
                        if tcq == 0:
                            nc.vector.tensor_add(
                                acc[:, KT:], pt0[:, KT:], pt[:, KT:]
                            )
                            nc.vector.tensor_copy(acc[:, 0:KT], pt0[:, 0:KT])
                        else:
                            nc.vector.tensor_add(acc[:], pt0[:], pt[:])
                    else:
                        nc.vector.tensor_add(
                            acc[:, off:], acc[:, off:], pt[:, off:]
                        )
                    nc.tensor.matmul(
                        pv[:, off:],
                        lhsT=v_sb[:, kt, h * HD : (h + 1) * HD],
                        rhs=pt[:, off:],
                        start=(kt == 0),
                        stop=(kt == nkt - 1),
                    )
                    if kt == 0:
                        flush_tail()

                def tail():
                    accb = small_pool.tile([128, CHK], BF16, tag="accb")
                    nc.vector.tensor_copy(accb[:], acc[:])
                    lb = ps_sc.tile([128, CHK], F32, tag="sc")
                    nc.tensor.matmul(
                        lb[:], lhsT=ones_sb[:], rhs=accb[:], start=True, stop=True
                    )
                    rbs = small_pool.tile([128, CHK], F32, tag="rbs")
                    nc.vector.reciprocal_approx_fast(rbs[:], lb[:])
                    aon = small_pool.tile([128, CHK], BF16, tag="aon")
                    nc.vector.tensor_mul(aon[:], pv[:], rbs[:])
                    dstt = a2a_in[b][tcq // 2]
                    for i in range(4):
                        d = 4 * (tcq % 2) + i
                        nc.sync.dma_start(
                            dstt[d, h], aon[:, i * KT : (i + 1) * KT]
                        )

                pending.append(tail)

            def emit_a2a(b, hf):
                flush_tail()
                nc.gpsimd.collective_compute(
                    "AllToAll",
                    mybir.AluOpType.bypass,
                    replica_groups=[list(range(NCORES))],
                    ins=[a2a_in[b][hf].opt()],
                    outs=[a2a_out[b][hf].opt()],
                )

            # ---------------- stage 2 ----------------
            gt_tiles = {}

            def load_gt(b, hf):
                # one bulk DMA on gpsimd, right after the producing collective:
                # ordering on the gpsimd stream guarantees it waits exactly for
                # its own A2A, and PE only waits on this DMA's semaphore.
                g = s2g_pool.tile(
                    [128, NDT, 128], BF16, tag="gt", bufs=2, name=f"g{b}{hf}"
                )
                src = a2a_out[b][hf][:].rearrange("s l p t -> p (s l) t")
                nc.gpsimd.dma_start(g[:], src)
                gt_tiles[(b, hf)] = g

            def emit_s2_piece(b, hf, oc):
                g = gt_tiles[(b, hf)]
                ps2 = ps_s2.tile([128, 512], F32, tag="s2")
                for ad in range(NDT):
                    nc.tensor.matmul(
                        ps2[:],
                        lhsT=g[:, ad, :],
                        rhs=wo_sb[:, ad, oc * 512 : (oc + 1) * 512],
                        start=(ad == 0),
                        stop=(ad == NDT - 1),
                    )
                ost = ost_pool.tile([128, 512], F32, tag="ost")
                nc.vector.tensor_copy(ost[:], ps2[:])
                r0 = b * 256 + hf * 128
                nc.sync.dma_start(
                    out.ap()[r0 : r0 + 128, oc * 512 : (oc + 1) * 512], ost[:]
                )

            # ---------------- schedule ----------------
            emit_initial_dmas()
            open_qkv(0)
            # only 5 s2 pieces run inside the chunk loop; the rest are held
            # back to fill the last A2A's ~30us flight in the tail.
            s2_sched = {0: [(0, 0, 0)], 1: [(0, 0, 1)], 2: [(0, 0, 2)],
                        3: [(0, 0, 3), (0, 1, 0)]}
            for i, (b, ch) in enumerate(chunk_list):
                if b == 1 and ch == 0:
                    open_qkv(1)
                if i + 1 < len(chunk_list):
                    issue_x_dma(i + 1)
                emit_wo_piece()
                emit_qkv_qk(b, ch, 0)
                emit_wo_piece()
                emit_qkv_qk(b, ch, 1)
                emit_qkv_v(b, ch)
                emit_attn(b, 0, ch)
                if b == 1 and s2_sched[ch]:
                    emit_s2_piece(*s2_sched[ch][0])
                emit_attn(b, 1, ch)
                if b == 1 and len(s2_sched[ch]) > 1:
                    emit_s2_piece(*s2_sched[ch][1])
                if ch == 1:
                    emit_a2a(b, 0)
                    load_gt(b, 0)
                elif ch == 3:
                    emit_a2a(b, 1)
                    load_gt(b, 1)
            # tail: the 7 deferred A2A(1,1)-independent pieces cover its flight
            for oc in range(1, 4):
                emit_s2_piece(0, 1, oc)
            for oc in range(4):
                emit_s2_piece(1, 0, oc)
            for oc in range(4):
                emit_s2_piece(1, 1, oc)

    nc.compile()
    return nc


_CACHED = {}


def _get_compiled():
    if "nc" not in _CACHED:
        _CACHED["nc"] = build_kernel()
    return _CACHED["nc"]


def shard_inputs(x, wq, wk, wv, wo, freqs_cos, freqs_sin, mask):
    x = np.asarray(x, np.float32)
    wq = np.asarray(wq, np.float32)
    wk = np.asarray(wk, np.float32)
    wv = np.asarray(wv, np.float32)
    wo = np.asarray(wo, np.float32)
    fc = np.asarray(freqs_cos, np.float32)
    fs = np.asarray(freqs_sin, np.float32)

    X = x.reshape(T, D).T  # [D, T]
    xt = (
        np.ascontiguousarray(X.reshape(NDT, 128, B * NCHK_B, CHK).transpose(1, 2, 0, 3))
        .astype(NP_BF16)
    )  # [128, 8, 16, 512]

    # de-interleave within each head: [0,2,...,126, 1,3,...,127]
    perm = np.concatenate([np.arange(0, HD, 2), np.arange(1, HD, 2)])

    ct = fc.T  # [64, S]
    st = fs.T
    cs1 = np.ascontiguousarray(np.concatenate([ct, -st], axis=0)).astype(NP_BF16)
    cs2 = np.ascontiguousarray(np.concatenate([st, ct], axis=0)).astype(NP_BF16)
    tri = np.triu(np.ones((128, 128), np.float32)).astype(NP_BF16)
    ones = np.ones((128, 128), np.float32).astype(NP_BF16)

    def tile_w(w_c):  # [256 out, D in] -> [128, NDT, 256]
        return np.ascontiguousarray(
            w_c.T.reshape(NDT, 128, 256).transpose(1, 0, 2)
        ).astype(NP_BF16)

    woT = wo.T  # [d_in, d_out]
    wot = np.ascontiguousarray(woT.reshape(NDT, 128, D).transpose(1, 0, 2)).astype(
        NP_BF16
    )

    in_maps = []
    for c in range(NCORES):
        rows = slice(c * HLOC * HD, (c + 1) * HLOC * HD)
        wq_c = wq[rows].reshape(HLOC, HD, D)[:, perm, :].reshape(HLOC * HD, D)
        wk_c = wk[rows].reshape(HLOC, HD, D)[:, perm, :].reshape(HLOC * HD, D)
        wv_c = wv[rows]
        in_maps.append(
            {
                "xt": xt,
                "wqp": tile_w(wq_c),
                "wkp": tile_w(wk_c),
                "wvp": tile_w(wv_c),
                "wop": wot,
                "cs1": cs1,
                "cs2": cs2,
                "trid": tri,
                "onesd": ones,
            }
        )
    return in_maps


def run_sharded(in_maps, trace=False):
    nc = _get_compiled()
    res = bass_utils.run_bass_kernel_spmd(
        nc, in_maps, core_ids=list(range(NCORES)), trace=trace
    )
    return res


def unshard(results):
    # results: list of dicts with "out": [512, D]; row = b*256 + half*128 + t
    A = np.stack([r["out"] for r in results])  # [8, 512, 2048]
    full = A.reshape(NCORES, B, 2, 128, D).transpose(1, 2, 0, 3, 4)
    return np.ascontiguousarray(full.reshape(B, S, D))


def kernel(**inputs):
    in_maps = shard_inputs(**inputs)
    res = run_sharded(in_maps, trace=False)
    return unshard(res.results)
